# revision 1
# baseline (speedup 1.0000x reference)
"""Trainium2 Bass kernel for nn_Attention (B=1, C=64, 12x12x12 spatial, 32 heads, head_dim=2).

Sharding: 32 heads split across 8 cores (4 heads/core). Each core computes
qkv projection for its heads, head-local attention (flash-style: S^T chunks
-> exp on ScalarE -> U/Z accumulation via matmul with V'=[V,1]), divides,
then applies its slice of w_proj rows to produce a partial output summed on
the host (tensor-parallel unshard) with bias/8 folded per core.

Uses bacc.Bacc (not plain Bass): its compile() runs
move_matmul_waits_to_ldweights + generate_event_semaphores, which the
TRN2 one-wait-per-instruction ISA constraint requires for Tile kernels.

Scheduling notes: Tile's static scheduler keeps per-engine creation order,
so overlap is structured by emission order — qkv for the second query tile
is emitted inside the first tile's key loop (PE is idle there; ScalarE exp
is the bottleneck), and the first tile's divide/proj are emitted before the
second tile's loop so they run under it.

Self-contained: hardcodes all shapes.
"""

import numpy as np
import ml_dtypes

import concourse.bass as bass
import concourse.bacc as bacc
import concourse.mybir as mybir
from concourse import tile
from concourse.bass_utils import run_bass_kernel_spmd

C = 64
N = 1728  # 12*12*12
NCORES = 8
HLOC = 4          # heads per core
SCALE = float(2.0 ** -0.5)

# key chunks: 13x128 + 64
KCS = [(i * 128, 128) for i in range(13)] + [(1664, 64)]
NKC = len(KCS)
# query tiles: big first tile, small second so the un-overlappable tail
# (reciprocal is FD-bound at 8 cyc/elem) is short
QTS = [(0, 1024), (1024, 704)]
# token chunks for proj; chunks 0..8 lie fully inside query tile 0
TCS = [(i * 108, 108) for i in range(16)]
TC_SPLIT = 9

F32 = mybir.dt.float32
BF16 = mybir.dt.bfloat16


def _sub_mms(qn):
    out = []
    o = 0
    while o < qn:
        n = min(512, qn - o)
        out.append((o, n))
        o += n
    return out


def build_nc():
    nc = bacc.Bacc(None)

    x2 = nc.declare_dram_parameter("x2", [C, N], BF16, isOutput=False)
    wq = nc.declare_dram_parameter("wq", [C, 2 * HLOC], BF16, isOutput=False)
    wk = nc.declare_dram_parameter("wk", [C, 2 * HLOC], BF16, isOutput=False)
    wv = nc.declare_dram_parameter("wv", [C, 2 * HLOC], BF16, isOutput=False)
    wp = nc.declare_dram_parameter("wp", [2 * HLOC + 1, C], F32, isOutput=False)
    y = nc.declare_dram_parameter("y", [N, C], F32, isOutput=True)

    with tile.TileContext(nc) as tc:
        with (
            tc.tile_pool(name="const", bufs=1) as cpool,
            tc.tile_pool(name="epool", bufs=5) as epool,
            tc.tile_pool(name="upool", bufs=2) as upool,
            tc.tile_pool(name="ps_s", bufs=3, space=bass.MemorySpace.PSUM) as ps_s,
            tc.tile_pool(name="ps_u", bufs=1, space=bass.MemorySpace.PSUM) as ps_u,
        ):
            x_st = cpool.tile([C, N], BF16, name="x_st")
            x_sb = cpool.tile([C, N], BF16, name="x_sb")
            wq_st = cpool.tile([C, 2 * HLOC], BF16, name="wq_st")
            wq_sb = cpool.tile([C, 2 * HLOC], BF16, name="wq_sb")
            wk_st = cpool.tile([C, 2 * HLOC], BF16, name="wk_st")
            wk_sb = cpool.tile([C, 2 * HLOC], BF16, name="wk_sb")
            wv_st = cpool.tile([C, 2 * HLOC], BF16, name="wv_st")
            wv_sb = cpool.tile([C, 2 * HLOC], BF16, name="wv_sb")
            wp_st = cpool.tile([2 * HLOC + 1, C], F32, name="wp_st")
            wp_sb = cpool.tile([2 * HLOC + 1, C], F32, name="wp_sb")
            qT = cpool.tile([128, N], BF16, name="qT")
            kT = cpool.tile([128, N], BF16, name="kT")
            vp = cpool.tile([128, NKC * 3 * HLOC], BF16, name="vp")
            ot = cpool.tile([2 * HLOC + 1, N], F32, name="ot")
            ybig = cpool.tile([128, len(TCS) * C], F32, name="ybig")
            ybv = ybig[:].rearrange("p (t c) -> p t c", c=C)

            # x DMA first (it gates everything); stage through one DVE copy
            # each so consumers wait on a single engine semaphore instead of
            # one per DMA queue.
            nc.sync.dma_start(out=x_st[:], in_=x2[:])
            nc.sync.dma_start(out=wv_st[:], in_=wv[:])
            nc.sync.dma_start(out=wq_st[:], in_=wq[:])
            nc.sync.dma_start(out=wk_st[:], in_=wk[:])
            nc.sync.dma_start(out=wp_st[:], in_=wp[:])
            nc.vector.tensor_copy(x_sb[:, 0:1024], x_st[:, 0:1024])
            nc.vector.tensor_copy(x_sb[:, 1024:N], x_st[:, 1024:N])
            nc.vector.tensor_copy(wv_sb[:], wv_st[:])
            nc.vector.tensor_copy(wq_sb[:], wq_st[:])
            nc.vector.tensor_copy(wk_sb[:], wk_st[:])
            nc.vector.tensor_copy(wp_sb[:], wp_st[:])

            # ones row for proj bias (rows 0..7 overwritten by attention out)
            nc.gpsimd.memset(ot[:, :], 1.0)
            # ones column per head in V' ([128, kc, h, 3], col 2 = 1.0)
            vp_v = vp[:].rearrange("p (a b c) -> p a b c", b=HLOC, c=3)
            nc.gpsimd.memset(vp_v[:, :, :, 2:3], 1.0)

            # ---- V': all 14 key chunks' V rows packed into ONE psum tile
            # (emitted as pre_u of kc0 — only U matmuls need it) ----
            def emit_vprime():
                psv = ps_s.tile([128, 1024], F32, tag="s", name="ps_v")
                for kc, (ko, kn) in enumerate(KCS):
                    nc.tensor.matmul(
                        psv[:kn, 8 * kc : 8 * kc + 2 * HLOC],
                        x_sb[:, ko : ko + kn],
                        wv_sb[:, 0 : 2 * HLOC],
                        start=True, stop=True,
                    )
                vsrc = psv[:, 0 : 8 * NKC].rearrange(
                    "p (kc h d) -> p kc h d", h=HLOC, d=2
                )
                nc.vector.tensor_copy(vp_v[:, :, :, 0:2], vsrc)

            def qkv_tile(w_sb, dst, off, qn, heads=range(HLOC)):
                """Per-head matmuls (rows at partitions 32h) + per-head copy."""
                ps = ps_s.tile([128, 1024], F32, tag="s", name="ps_qkv")
                for h in heads:
                    for (o, n_) in _sub_mms(qn):
                        nc.tensor.matmul(
                            ps[32 * h : 32 * h + 2, o : o + n_],
                            w_sb[:, 2 * h : 2 * h + 2],
                            x_sb[:, off + o : off + o + n_],
                            start=True, stop=True,
                            tile_position=(0, 32 * h),
                        )
                    nc.vector.tensor_copy(
                        dst[32 * h : 32 * h + 2, off : off + qn],
                        ps[32 * h : 32 * h + 2, :qn],
                    )

            # q half 0 / first k cols are emitted per-head just before each
            # head's first S matmul (pre_s of kc0) so exp h0 starts ASAP
            def pre_s0(kc, h):
                if kc == 0:
                    qkv_tile(wq_sb, qT, 0, 1024, heads=[h])
                    qkv_tile(wk_sb, kT, 0, 512, heads=[h])

            def pre_u0(kc):
                if kc == 0:
                    emit_vprime()

            def divide_and_store(pu, qo, qn, last=False):
                """O^T rows 2h+d of `ot` <- U rows / Z row (per head)."""
                if last:
                    # final tile: read PSUM directly, no next user of the slot
                    usrc = pu[:, :qn]
                else:
                    u_sb = upool.tile([128, 1024], F32, tag="u_sb", name="u_sb")
                    nc.vector.tensor_copy(u_sb[:, :qn], pu[:, :qn])
                    usrc = u_sb[:, :qn]
                zrec = upool.tile([128, 1024], F32, tag="zrec", name="zrec")
                nc.vector.reciprocal(zrec[:, :qn], usrc)
                zz = upool.tile([128, 1024], F32, tag="zz", name="zz")
                zzv_ = zz[:, :qn].rearrange("(h g) f -> h g f", g=32)
                zrv_ = zrec[:, :qn].rearrange("(h g) f -> h g f", g=32)
                nc.sync.dma_start(out=zzv_[:, 0, :], in_=zrv_[:, 2, :])
                nc.gpsimd.dma_start(out=zzv_[:, 1, :], in_=zrv_[:, 2, :])
                osp = upool.tile([128, 1024], F32, tag="osp", name="osp")
                nc.vector.tensor_mul(osp[:, :qn], usrc, zz[:, :qn])
                ospv = osp[:, :qn].rearrange("(h g) f -> h g f", g=32)
                otv = ot[0 : 2 * HLOC, qo : qo + qn].rearrange("(h g) f -> h g f", g=2)
                nc.sync.dma_start(out=otv[:, 0, :], in_=ospv[:, 0, :])
                nc.gpsimd.dma_start(out=otv[:, 1, :], in_=ospv[:, 1, :])

            def proj_chunks(ts_):
                for t in ts_:
                    to, tn = TCS[t]
                    py = ps_s.tile([128, 1024], F32, tag="s", name="py")
                    nc.tensor.matmul(
                        py[:tn, 0:C], ot[:, to : to + tn], wp_sb[:],
                        start=True, stop=True,
                    )
                    nc.vector.tensor_copy(ybv[:tn, t, :], py[:tn, 0:C])

            def main_loop(qo, qn, boundary_work, pre_s=None, pre_u=None):
                pu = ps_u.tile([128, 1024], F32, tag="pu", name="pu")
                for kc, (ko, kn) in enumerate(KCS):
                    es = []
                    for h in range(HLOC):
                        if pre_s is not None:
                            pre_s(kc, h)
                        ps = ps_s.tile([128, 1024], F32, tag="s", name="ps_att")
                        for (o, n_) in _sub_mms(qn):
                            nc.tensor.matmul(
                                ps[:kn, o : o + n_],
                                kT[32 * h : 32 * h + 2, ko : ko + kn],
                                qT[32 * h : 32 * h + 2, qo + o : qo + o + n_],
                                start=True, stop=True,
                                tile_position=(32 * h, 0),
                            )
                        e = epool.tile([128, 1024], BF16, tag="e", name="e")
                        nc.scalar.activation(
                            e[:kn, :qn], ps[:kn, :qn],
                            mybir.ActivationFunctionType.Exp, scale=SCALE,
                        )
                        es.append(e)
                    if pre_u is not None:
                        pre_u(kc)
                    for h in range(HLOC):
                        for (o, n_) in _sub_mms(qn):
                            nc.tensor.matmul(
                                pu[32 * h : 32 * h + 3, o : o + n_],
                                vp_v[:kn, kc, h, :],
                                es[h][:kn, o : o + n_],
                                start=(kc == 0), stop=(kc == NKC - 1),
                                tile_position=(0, 32 * h),
                            )
                    work = boundary_work.get(kc)
                    if work:
                        work()
                return pu

            # qt0 loop: remaining qkv emitted at key-loop boundaries, one
            # small piece per boundary (PE slack under the ACT-bound loop)
            bw0 = {
                0: lambda: qkv_tile(wk_sb, kT, 512, 512),
                1: lambda: qkv_tile(wq_sb, qT, 1024, 704, heads=[0, 1]),
                2: lambda: qkv_tile(wq_sb, qT, 1024, 704, heads=[2, 3]),
                3: lambda: qkv_tile(wk_sb, kT, 1024, 704, heads=[0, 1]),
                4: lambda: qkv_tile(wk_sb, kT, 1024, 704, heads=[2, 3]),
            }
            pu0 = main_loop(0, 1024, bw0, pre_s=pre_s0, pre_u=pre_u0)
            divide_and_store(pu0, 0, 1024)

            # qt1 loop: qt0's proj + first y DMA emitted at late boundaries
            # (after qt0's divide chain has drained on DVE/DMA)
            def y_dma0():
                yv0 = y[0 : TC_SPLIT * 108, :].rearrange("(t i) c -> i t c", i=108)
                nc.sync.dma_start(out=yv0, in_=ybv[:108, 0:TC_SPLIT, :])

            bw1 = {kc: (lambda t=kc - 3: proj_chunks([t])) for kc in range(3, 12)}
            bw1[12] = y_dma0
            pu1 = main_loop(1024, 704, bw1)
            divide_and_store(pu1, 1024, 704, last=True)
            proj_chunks(range(TC_SPLIT, len(TCS)))
            yv1 = y[TC_SPLIT * 108 :, :].rearrange("(t i) c -> i t c", i=108)
            nc.sync.dma_start(out=yv1, in_=ybv[:108, TC_SPLIT:, :])

    return nc


_NC = None


def _get_nc():
    global _NC
    if _NC is None:
        _NC = build_nc()
        _NC.finalize()
    return _NC


def make_in_maps(x, w_qkv, w_proj, b_proj):
    x2 = np.ascontiguousarray(x.reshape(C, N)).astype(ml_dtypes.bfloat16)
    in_maps = []
    for c in range(NCORES):
        sl = slice(8 * c, 8 * c + 8)
        wq = np.ascontiguousarray(w_qkv[sl, :].T).astype(ml_dtypes.bfloat16)
        wk = np.ascontiguousarray(w_qkv[64 + 8 * c : 64 + 8 * c + 8, :].T).astype(
            ml_dtypes.bfloat16
        )
        wv = np.ascontiguousarray(w_qkv[128 + 8 * c : 128 + 8 * c + 8, :].T).astype(
            ml_dtypes.bfloat16
        )
        wp = np.concatenate(
            [w_proj[:, sl].T, (b_proj / NCORES)[None, :]], axis=0
        ).astype(np.float32)
        in_maps.append(
            {"x2": x2, "wq": wq, "wk": wk, "wv": wv, "wp": np.ascontiguousarray(wp)}
        )
    return in_maps


def run(x, w_qkv, w_proj, b_proj, trace=False, **kw):
    nc = _get_nc()
    in_maps = make_in_maps(x, w_qkv, w_proj, b_proj)
    res = run_bass_kernel_spmd(
        nc, in_maps, core_ids=list(range(NCORES)), trace=trace, **kw
    )
    y = np.zeros((N, C), np.float32)
    for r in res.results:
        y += r["y"]
    return y.reshape(1, 12, 12, 12, C), res


def kernel(x, w_qkv, w_proj, b_proj):
    out, _ = run(
        np.asarray(x), np.asarray(w_qkv), np.asarray(w_proj), np.asarray(b_proj)
    )
    return out



# revision 9
# speedup vs baseline: 1.1873x; 1.1873x over previous
"""Trainium2 Bass kernel for nn_Attention (B=1, C=64, 12x12x12 spatial, 32 heads, head_dim=2).

Sharding: 32 heads split across 8 cores (4 heads/core), host sums the
8 partial w_proj outputs (tensor-parallel unshard, bias/8 per core).

Core design (v2, ACT-bound at ~91us of exp):
- Query blocks of (512,512,512,192), key chunks 13x128 + 64.
- One exp ACTIVATE per (block, chunk) covering all 4 heads (F=2048 from
  4 PSUM banks) -> amortizes the ~290-cycle per-instruction ACT overhead.
- PSUM managed manually as one [128,4096] tile: two 4-bank S buffers
  ping-pong (even/odd chunk). U_chunk matmuls are carved into bank 3 of
  the buffer ACT just finished (h3's region -- the LAST S matmul of the
  next same-parity chunk to touch it, so the DVE drain hides), proj and
  qkv staging into bank 2. DVE accumulates U into SBUF (u_acc) so no
  PSUM bank persists across the chunk loop.
- Software pipelining: S(kc+1) is emitted BEFORE U(kc) -- PE's queue is
  strict in-order, so the baseline's order (U before next S) serialized
  exp(kc) -> U(kc) -> S(kc+1) -> exp(kc+1) and starved ACT.
- Tail key chunk (64 keys) packs head pairs on partitions (rows 0:64 /
  64:128) halving its exp free-size; U uses block-diagonal V' weights.
- Per-head qkv matmuls run as a dense PE burst at t0 (warms the PE HAM
  clock gate; cold 1.2GHz PE was half the baseline's loss) and continue
  as carved pieces at chunk boundaries.
- Dummy 8-elem exp at t0 pulls the ~2.7us ACT table load under the
  input DMA. Divide uses reciprocal_approx_fast; divide+proj of block b
  run under block b+1's chunk loop so only the 192-block drains at the
  end.
"""

import numpy as np
import ml_dtypes

import concourse.bass as bass
import concourse.bacc as bacc
import concourse.mybir as mybir
from concourse import tile
from concourse.bass_utils import run_bass_kernel_spmd

C = 64
N = 1728
NCORES = 8
HLOC = 4
SCALE = float(2.0 ** -0.5)

KCS = [(i * 128, 128) for i in range(13)] + [(1664, 64)]
NKC = len(KCS)
QB = [(0, 512), (512, 512), (1024, 512), (1536, 192)]

F32 = mybir.dt.float32
BF16 = mybir.dt.bfloat16
EXPF = mybir.ActivationFunctionType.Exp


def build_nc():
    nc = bacc.Bacc(None)

    x2 = nc.declare_dram_parameter("x2", [C, N], BF16, isOutput=False)
    wq = nc.declare_dram_parameter("wq", [C, 2 * HLOC], BF16, isOutput=False)
    wk = nc.declare_dram_parameter("wk", [C, 2 * HLOC], BF16, isOutput=False)
    wv = nc.declare_dram_parameter("wv", [C, 2 * HLOC], BF16, isOutput=False)
    wp = nc.declare_dram_parameter("wp", [2 * HLOC + 1, C], F32, isOutput=False)
    y = nc.declare_dram_parameter("y", [N, C], F32, isOutput=True)

    with tile.TileContext(nc) as tc:
        with (
            tc.tile_pool(name="const", bufs=1) as cpool,
            tc.tile_pool(name="epool", bufs=3) as epool,
            tc.tile_pool(name="ps", bufs=1, space=bass.MemorySpace.PSUM) as pspool,
        ):
            x_sb = cpool.tile([C, N], BF16, name="x_sb")
            wq_sb = cpool.tile([C, 2 * HLOC], BF16, name="wq_sb")
            wk_sb = cpool.tile([C, 2 * HLOC], BF16, name="wk_sb")
            wv_sb = cpool.tile([C, 2 * HLOC], BF16, name="wv_sb")
            wp_sb = cpool.tile([2 * HLOC + 1, C], F32, name="wp_sb")
            qT = cpool.tile([128, N], BF16, name="qT")
            kT = cpool.tile([128, N], BF16, name="kT")
            vp = cpool.tile([128, NKC * HLOC * 3], BF16, name="vp")
            vpA = cpool.tile([128, 35], BF16, name="vpA")
            vpB = cpool.tile([128, 35], BF16, name="vpB")
            u_acc = cpool.tile([128, N], F32, name="u_acc")
            zrec = cpool.tile([128, 512], F32, name="zrec")
            osp = cpool.tile([128, 512], F32, name="osp")
            ot = cpool.tile([16, N], F32, name="ot")
            ybig = cpool.tile([128, 14 * C], F32, name="ybig")
            dum = cpool.tile([1, 16], F32, name="dum")
            PS = pspool.tile([128, 4096], F32, name="PS")

            vp_v = vp[:].rearrange("p (kc h d) -> p kc h d", h=HLOC, d=3)
            ybig_v = ybig[:].rearrange("p (t c) -> p t c", c=C)

            def hg(t, c0, c1, r0, r1, g=32):
                """Partitions {g*h + r0..r1}, cols c0..c1 -> [4, r, c] view."""
                return t[:, c0:c1].rearrange("(h g) f -> h g f", g=g)[:, r0:r1, :]

            # ---- t0: input DMAs, memsets, ACT table prefetch ----
            nc.sync.dma_start(out=x_sb[:, 0:864], in_=x2[:, 0:864])
            nc.scalar.dma_start(out=x_sb[:, 864:N], in_=x2[:, 864:N])
            nc.gpsimd.dma_start(out=wq_sb[:], in_=wq[:])
            nc.gpsimd.dma_start(out=wk_sb[:], in_=wk[:])
            nc.gpsimd.dma_start(out=wv_sb[:], in_=wv[:])
            nc.gpsimd.dma_start(out=wp_sb[:], in_=wp[:])
            nc.gpsimd.memset(dum[:], 1.0)
            nc.gpsimd.memset(ot[:, :], 1.0)
            nc.gpsimd.memset(vp_v[:, :, :, 2:3], 1.0)
            nc.gpsimd.memset(vpA[:], 0.0)
            nc.gpsimd.memset(vpB[:], 0.0)
            nc.scalar.activation(dum[0:1, 8:16], dum[0:1, 0:8], EXPF)

            # ---- helpers ----
            def qk_piece(w_sb, dst, o, w, creg):
                """q or k for all 4 heads over x cols [o, o+w) via psum carve."""
                for h in range(HLOC):
                    nc.tensor.matmul(
                        PS[32 * h : 32 * h + 2, creg : creg + w],
                        w_sb[:, 2 * h : 2 * h + 2],
                        x_sb[:, o : o + w],
                        start=True, stop=True,
                        tile_position=(0, 32 * h),
                    )
                # contiguous partitions (DVE can't stride the partition dim);
                # junk rows between head groups land in unused qT/kT rows
                nc.vector.tensor_copy(
                    dst[0:98, o : o + w], PS[0:98, creg : creg + w]
                )

            def vprime():
                for kc, (ko, kn) in enumerate(KCS):
                    nc.tensor.matmul(
                        PS[0:kn, 2048 + 8 * kc : 2048 + 8 * kc + 8],
                        x_sb[:, ko : ko + kn],
                        wv_sb[:, 0 : 2 * HLOC],
                        start=True, stop=True,
                    )
                vsrc = PS[:, 2048 : 2048 + 8 * NKC].rearrange(
                    "p (kc h d) -> p kc h d", h=HLOC, d=2
                )
                nc.vector.tensor_copy(vp_v[:, :, :, 0:2], vsrc)
                # tail-pair U weights: (v0,v1,1) rows; h1/h3 shifted to
                # partitions 64:128 via DMA (cross-partition move)
                nc.sync.dma_start(out=vpA[0:64, 0:3], in_=vp_v[0:64, 13, 0, :])
                nc.gpsimd.dma_start(out=vpA[64:128, 32:35], in_=vp_v[0:64, 13, 1, :])
                nc.sync.dma_start(out=vpB[0:64, 0:3], in_=vp_v[0:64, 13, 2, :])
                nc.gpsimd.dma_start(out=vpB[64:128, 32:35], in_=vp_v[0:64, 13, 3, :])

            def emit_S(b, kc):
                qo, qn = QB[b]
                ko, kn = KCS[kc]
                buf = 0 if kc % 2 == 0 else 2048
                if kc < 13:
                    for h in range(HLOC):
                        nc.tensor.matmul(
                            PS[0:kn, buf + 512 * h : buf + 512 * h + qn],
                            kT[32 * h : 32 * h + 2, ko : ko + kn],
                            qT[32 * h : 32 * h + 2, qo : qo + qn],
                            start=True, stop=True,
                            tile_position=(32 * h, 0),
                        )
                else:
                    for h in range(HLOC):
                        pr, pc = 64 * (h % 2), 512 * (h // 2)
                        nc.tensor.matmul(
                            PS[pr : pr + 64, buf + pc : buf + pc + qn],
                            kT[32 * h : 32 * h + 2, ko : ko + kn],
                            qT[32 * h : 32 * h + 2, qo : qo + qn],
                            start=True, stop=True,
                            tile_position=(32 * h, pr),
                        )

            def emit_exp(b, kc):
                qo, qn = QB[b]
                ko, kn = KCS[kc]
                buf = 0 if kc % 2 == 0 else 2048
                et = epool.tile([128, 2048], BF16, tag="e", name="et")
                if kc < 13:
                    if qn == 512:
                        nc.scalar.activation(
                            et[0:kn, 0:2048], PS[0:kn, buf : buf + 2048],
                            EXPF, scale=SCALE,
                        )
                    else:
                        src = PS[0:kn, buf : buf + 2048].rearrange(
                            "p (h q) -> p h q", h=4
                        )[:, :, 0:qn]
                        dst = et[0:kn, 0 : 4 * qn].rearrange("p (h q) -> p h q", h=4)
                        nc.scalar.activation(dst, src, EXPF, scale=SCALE)
                else:
                    if qn == 512:
                        nc.scalar.activation(
                            et[:, 0:1024], PS[:, buf : buf + 1024],
                            EXPF, scale=SCALE,
                        )
                    else:
                        src = PS[:, buf : buf + 1024].rearrange(
                            "p (a q) -> p a q", a=2
                        )[:, :, 0:qn]
                        dst = et[:, 0 : 2 * qn].rearrange("p (a q) -> p a q", a=2)
                        nc.scalar.activation(dst, src, EXPF, scale=SCALE)
                return et

            def emit_U_add(b, kc, et):
                qo, qn = QB[b]
                ko, kn = KCS[kc]
                buf = 0 if kc % 2 == 0 else 2048
                cv = buf + 1536  # carve: bank 3 of the freed buffer
                if kc < 13:
                    for h in range(HLOC):
                        nc.tensor.matmul(
                            PS[32 * h : 32 * h + 3, cv : cv + qn],
                            vp_v[0:kn, kc, h, :],
                            et[0:kn, qn * h : qn * h + qn],
                            start=True, stop=True,
                            tile_position=(0, 32 * h),
                        )
                else:
                    nc.tensor.matmul(
                        PS[0:35, cv : cv + qn], vpA[:], et[:, 0:qn],
                        start=True, stop=True, tile_position=(0, 0),
                    )
                    nc.tensor.matmul(
                        PS[64:99, cv : cv + qn], vpB[:], et[:, qn : 2 * qn],
                        start=True, stop=True, tile_position=(0, 64),
                    )
                uc = PS[0:99, cv : cv + qn]
                ua = u_acc[0:99, qo : qo + qn]
                if kc == 0:
                    nc.vector.tensor_copy(ua, uc)
                else:
                    nc.vector.tensor_add(ua, ua, uc)

            def divide_piece(bprev, i, buf):
                qo, qn = QB[bprev]

                def uv(r0, r1):
                    return hg(u_acc, qo, qo + qn, r0, r1)

                def zr(r0, r1):
                    return hg(zrec, 0, qn, r0, r1)

                if i == 0:
                    nc.vector.reciprocal_approx_fast(
                        zrec[0:99, 0:qn], u_acc[0:99, qo : qo + qn]
                    )
                    nc.sync.dma_start(out=zr(0, 1), in_=zr(2, 3))
                    nc.gpsimd.dma_start(out=zr(1, 2), in_=zr(2, 3))
                elif i == 1:
                    nc.vector.tensor_mul(
                        osp[0:99, 0:qn], u_acc[0:99, qo : qo + qn], zrec[0:99, 0:qn]
                    )
                    for d, eng in ((0, nc.sync), (1, nc.gpsimd)):
                        otv = ot[0 : 2 * HLOC, qo : qo + qn].rearrange(
                            "(h g) f -> h g f", g=2
                        )[:, d : d + 1, :]
                        eng.dma_start(out=otv, in_=hg(osp, 0, qn, d, d + 1))
                elif 2 <= i <= 5:
                    j = qo // 128 + (i - 2)
                    if j * 128 >= qo + qn:
                        return
                    tn = min(128, qo + qn - j * 128)
                    nc.tensor.matmul(
                        PS[0:tn, buf + 1024 : buf + 1024 + C],
                        ot[0 : 2 * HLOC + 1, 128 * j : 128 * j + tn],
                        wp_sb[:],
                        start=True, stop=True,
                    )
                    nc.vector.tensor_copy(
                        ybig_v[0:tn, j, :], PS[0:tn, buf + 1024 : buf + 1024 + C]
                    )
                elif i == 6:
                    j0, nj = qo // 128, qn // 128
                    yv = y[qo : qo + 128 * nj, :].rearrange("(t i) c -> i t c", i=128)
                    nc.sync.dma_start(out=yv, in_=ybig_v[:, j0 : j0 + nj, :])

            def boundary(b, kc, buf):
                if b == 0:
                    if kc <= 9:  # k chunks 4..13 (last is 64 wide)
                        ko2, kw = KCS[kc + 4]
                        qk_piece(wk_sb, kT, ko2, kw, buf + 1024)
                    elif kc in (10, 11):  # q block1
                        qk_piece(wq_sb, qT, 512 + 256 * (kc - 10), 256, buf + 1024)
                else:
                    if 2 <= kc <= 8:
                        divide_piece(b - 1, kc - 2, buf)
                    if b == 1 and kc in (9, 10):  # q block2
                        qk_piece(wq_sb, qT, 1024 + 256 * (kc - 9), 256, buf + 1024)
                    if b == 2 and kc == 9:  # q block3 (192 wide)
                        qk_piece(wq_sb, qT, 1536, 192, buf + 1024)

            # ---- prologue PE burst (staged in bufB regions) ----
            qk_piece(wq_sb, qT, 0, 512, 2560)   # q block0 -> bufB bank1
            qk_piece(wk_sb, kT, 0, 128, 3584)   # k chunk0 -> bufB bank3
            emit_S(0, 0)
            vprime()                             # V' psv -> bufB bank0
            for i in range(1, 4):                # k chunks 1-3 -> bufB bank3
                qk_piece(wk_sb, kT, 128 * i, 128, 3584 + 128 * i)

            # ---- main loop (S software-pipelined one chunk ahead) ----
            for b in range(4):
                for kc in range(NKC):
                    buf = 0 if kc % 2 == 0 else 2048
                    et = emit_exp(b, kc)
                    if kc < 13:
                        emit_S(b, kc + 1)
                    elif b < 3:
                        emit_S(b + 1, 0)
                    emit_U_add(b, kc, et)
                    boundary(b, kc, buf)

            # ---- tail: divide + proj + store for the final 192-block ----
            for i in range(6):
                divide_piece(3, i, 0)
            nc.sync.dma_start(
                out=y[1536:1664, :].rearrange("(t i) c -> i t c", i=128),
                in_=ybig_v[:, 12:13, :],
            )
            nc.sync.dma_start(
                out=y[1664:1728, :].rearrange("(t i) c -> i t c", i=64),
                in_=ybig_v[0:64, 13:14, :],
            )

    return nc


_NC = None


def _get_nc():
    global _NC
    if _NC is None:
        _NC = build_nc()
        _NC.finalize()
    return _NC


def make_in_maps(x, w_qkv, w_proj, b_proj):
    x2 = np.ascontiguousarray(x.reshape(C, N)).astype(ml_dtypes.bfloat16)
    in_maps = []
    for c in range(NCORES):
        sl = slice(8 * c, 8 * c + 8)
        wq = np.ascontiguousarray(w_qkv[sl, :].T).astype(ml_dtypes.bfloat16)
        wk = np.ascontiguousarray(w_qkv[64 + 8 * c : 64 + 8 * c + 8, :].T).astype(
            ml_dtypes.bfloat16
        )
        wv = np.ascontiguousarray(w_qkv[128 + 8 * c : 128 + 8 * c + 8, :].T).astype(
            ml_dtypes.bfloat16
        )
        wp = np.concatenate(
            [w_proj[:, sl].T, (b_proj / NCORES)[None, :]], axis=0
        ).astype(np.float32)
        in_maps.append(
            {"x2": x2, "wq": wq, "wk": wk, "wv": wv, "wp": np.ascontiguousarray(wp)}
        )
    return in_maps


def run(x, w_qkv, w_proj, b_proj, trace=False, **kw):
    nc = _get_nc()
    in_maps = make_in_maps(x, w_qkv, w_proj, b_proj)
    res = run_bass_kernel_spmd(
        nc, in_maps, core_ids=list(range(NCORES)), trace=trace, **kw
    )
    y = np.zeros((N, C), np.float32)
    for r in res.results:
        y += r["y"]
    return y.reshape(1, 12, 12, 12, C), res


def kernel(x, w_qkv, w_proj, b_proj):
    out, _ = run(
        np.asarray(x), np.asarray(w_qkv), np.asarray(w_proj), np.asarray(b_proj)
    )
    return out


# revision 21
# speedup vs baseline: 1.3678x; 1.1520x over previous
"""Trainium2 Bass kernel for nn_Attention (B=1, C=64, 12x12x12 spatial, 32 heads, head_dim=2).

Sharding: 32 heads split across 8 cores (4 heads/core), host sums the
8 partial w_proj outputs (tensor-parallel unshard, bias/8 per core).

Core design (v2, ACT-bound at ~91us of exp):
- Query blocks of (512,512,512,192), key chunks 13x128 + 64.
- One exp ACTIVATE per (block, chunk) covering all 4 heads (F=2048 from
  4 PSUM banks) -> amortizes the ~290-cycle per-instruction ACT overhead.
- PSUM managed manually as one [128,4096] tile: two 4-bank S buffers
  ping-pong (even/odd chunk). U_chunk matmuls are carved into bank 3 of
  the buffer ACT just finished (h3's region -- the LAST S matmul of the
  next same-parity chunk to touch it, so the DVE drain hides), proj and
  qkv staging into bank 2. DVE accumulates U into SBUF (u_acc) so no
  PSUM bank persists across the chunk loop.
- Software pipelining: S(kc+1) is emitted BEFORE U(kc) -- PE's queue is
  strict in-order, so the baseline's order (U before next S) serialized
  exp(kc) -> U(kc) -> S(kc+1) -> exp(kc+1) and starved ACT.
- Tail key chunk (64 keys) packs head pairs on partitions (rows 0:64 /
  64:128) halving its exp free-size; U uses block-diagonal V' weights.
- Per-head qkv matmuls run as a dense PE burst at t0 (warms the PE HAM
  clock gate; cold 1.2GHz PE was half the baseline's loss) and continue
  as carved pieces at chunk boundaries.
- Dummy 8-elem exp at t0 pulls the ~2.7us ACT table load under the
  input DMA. Divide uses reciprocal_approx_fast; divide+proj of block b
  run under block b+1's chunk loop so only the 192-block drains at the
  end.
"""

import numpy as np
import ml_dtypes

import concourse.bass as bass
import concourse.bacc as bacc
import concourse.mybir as mybir
from concourse import tile
from concourse.bass_utils import run_bass_kernel_spmd

C = 64
N = 1728
NCORES = 8
HLOC = 4
SCALE = float(2.0 ** -0.5)

KCS = [(i * 128, 128) for i in range(13)] + [(1664, 64)]
NKC = len(KCS)
QB = [(0, 512), (512, 512), (1024, 512), (1536, 192)]

F32 = mybir.dt.float32
BF16 = mybir.dt.bfloat16
EXPF = mybir.ActivationFunctionType.Exp


def build_nc():
    nc = bacc.Bacc(None)

    x2 = nc.declare_dram_parameter("x2", [C, N], BF16, isOutput=False)
    wq = nc.declare_dram_parameter("wq", [C, 2 * HLOC], BF16, isOutput=False)
    wk = nc.declare_dram_parameter("wk", [C, 2 * HLOC], BF16, isOutput=False)
    wv = nc.declare_dram_parameter("wv", [C, 2 * HLOC], BF16, isOutput=False)
    wp = nc.declare_dram_parameter("wp", [2 * HLOC + 1, C], F32, isOutput=False)
    y = nc.declare_dram_parameter("y", [C, N], F32, isOutput=True)

    with tile.TileContext(nc) as tc:
        with (
            tc.tile_pool(name="const", bufs=1) as cpool,
            tc.tile_pool(name="epool", bufs=3) as epool,
            tc.tile_pool(name="ps", bufs=1, space=bass.MemorySpace.PSUM) as pspool,
        ):
            x_sb = cpool.tile([C, N], BF16, name="x_sb")
            wq_sb = cpool.tile([C, 2 * HLOC], BF16, name="wq_sb")
            wk_sb = cpool.tile([C, 2 * HLOC], BF16, name="wk_sb")
            wv_sb = cpool.tile([C, 2 * HLOC], BF16, name="wv_sb")
            wp_sb = cpool.tile([2 * HLOC + 1, C], F32, name="wp_sb")
            qT = cpool.tile([128, N], BF16, name="qT")
            kT = cpool.tile([128, N], BF16, name="kT")
            vp = cpool.tile([128, NKC * HLOC * 3], BF16, name="vp")
            u_acc = cpool.tile([128, N], F32, name="u_acc")
            zrec = cpool.tile([128, 512], F32, name="zrec")
            osp = cpool.tile([128, 512], F32, name="osp")
            ot = cpool.tile([16, N], F32, name="ot")
            ySB = cpool.tile([C, N], F32, name="ySB")
            dum = cpool.tile([1, 16], F32, name="dum")
            PS = pspool.tile([128, 4096], F32, name="PS")

            vp_v = vp[:].rearrange("p (kc h d) -> p kc h d", h=HLOC, d=3)

            def hg(t, c0, c1, r0, r1, g=32):
                """Partitions {g*h + r0..r1}, cols c0..c1 -> [4, r, c] view."""
                return t[:, c0:c1].rearrange("(h g) f -> h g f", g=g)[:, r0:r1, :]

            # ---- t0: ACT table prefetch first (nothing on the ACT queue
            # before the dummy exp), input DMAs on sync (spread over HW
            # queues), weights on gpsimd SWDGE ----
            nc.vector.memset(dum[:], 1.0)
            nc.scalar.activation(dum[0:1, 8:16], dum[0:1, 0:8], EXPF)
            nc.sync.dma_start(out=x_sb[:, 0:576], in_=x2[:, 0:576])
            nc.sync.dma_start(out=x_sb[:, 576:1152], in_=x2[:, 576:1152])
            nc.sync.dma_start(out=x_sb[:, 1152:N], in_=x2[:, 1152:N])
            nc.gpsimd.dma_start(out=wq_sb[:], in_=wq[:])
            nc.gpsimd.dma_start(out=wk_sb[:], in_=wk[:])
            nc.gpsimd.dma_start(out=wv_sb[:], in_=wv[:])
            nc.gpsimd.dma_start(out=wp_sb[:], in_=wp[:])
            nc.gpsimd.memset(ot[:, :], 1.0)
            nc.gpsimd.memset(vp_v[:, :, :, 2:3], 1.0)

            # ---- helpers ----
            def qk_piece(w_sb, dst, o, w, creg):
                """q or k for all 4 heads over x cols [o, o+w) via psum carve."""
                for h in range(HLOC):
                    nc.tensor.matmul(
                        PS[32 * h : 32 * h + 2, creg : creg + w],
                        w_sb[:, 2 * h : 2 * h + 2],
                        x_sb[:, o : o + w],
                        start=True, stop=True,
                        tile_position=(0, 32 * h),
                    )
                # contiguous partitions (DVE can't stride the partition dim);
                # junk rows between head groups land in unused qT/kT rows
                nc.vector.tensor_copy(
                    dst[0:98, o : o + w], PS[0:98, creg : creg + w]
                )

            def vprime(k0, k1):
                for kc in range(k0, k1):
                    ko, kn = KCS[kc]
                    nc.tensor.matmul(
                        PS[0:kn, 2048 + 8 * kc : 2048 + 8 * kc + 8],
                        x_sb[:, ko : ko + kn],
                        wv_sb[:, 0 : 2 * HLOC],
                        start=True, stop=True,
                    )
                vsrc = PS[:, 2048 + 8 * k0 : 2048 + 8 * k1].rearrange(
                    "p (kc h d) -> p kc h d", h=HLOC, d=2
                )
                nc.vector.tensor_copy(vp_v[:, k0:k1, :, 0:2], vsrc)

            def emit_S(b, kc):
                qo, qn = QB[b]
                ko, kn = KCS[kc]
                buf = 0 if kc % 2 == 0 else 2048
                for h in range(HLOC):
                    nc.tensor.matmul(
                        PS[0:kn, buf + 512 * h : buf + 512 * h + qn],
                        kT[32 * h : 32 * h + 2, ko : ko + kn],
                        qT[32 * h : 32 * h + 2, qo : qo + qn],
                        start=True, stop=True,
                        tile_position=(32 * h, 0),
                    )

            def emit_exp(b, kc):
                qo, qn = QB[b]
                ko, kn = KCS[kc]
                buf = 0 if kc % 2 == 0 else 2048
                et = epool.tile([128, 2048], BF16, tag="e", name="et")
                if qn == 512:
                    nc.scalar.activation(
                        et[0:kn, 0:2048], PS[0:kn, buf : buf + 2048],
                        EXPF, scale=SCALE,
                    )
                else:
                    src = PS[0:kn, buf : buf + 2048].rearrange(
                        "p (h q) -> p h q", h=4
                    )[:, :, 0:qn]
                    dst = et[0:kn, 0 : 4 * qn].rearrange("p (h q) -> p h q", h=4)
                    nc.scalar.activation(dst, src, EXPF, scale=SCALE)
                return et

            def emit_U_add(b, kc, et):
                qo, qn = QB[b]
                ko, kn = KCS[kc]
                buf = 0 if kc % 2 == 0 else 2048
                cv = buf + 1536  # carve: bank 3 of the freed buffer
                for h in range(HLOC):
                    nc.tensor.matmul(
                        PS[32 * h : 32 * h + 3, cv : cv + qn],
                        vp_v[0:kn, kc, h, :],
                        et[0:kn, qn * h : qn * h + qn],
                        start=True, stop=True,
                        tile_position=(0, 32 * h),
                    )
                uc = PS[0:99, cv : cv + qn]
                ua = u_acc[0:99, qo : qo + qn]
                if kc == 0:
                    nc.vector.tensor_copy(ua, uc)
                else:
                    nc.vector.tensor_add(ua, ua, uc)

            def divide_piece(bprev, i, buf):
                qo, qn = QB[bprev]

                def uv(r0, r1):
                    return hg(u_acc, qo, qo + qn, r0, r1)

                def zr(r0, r1):
                    return hg(zrec, 0, qn, r0, r1)

                if i == 0:
                    nc.vector.reciprocal_approx_fast(
                        zrec[0:99, 0:qn], u_acc[0:99, qo : qo + qn]
                    )
                    nc.sync.dma_start(out=zr(0, 1), in_=zr(2, 3))
                    nc.sync.dma_start(out=zr(1, 2), in_=zr(2, 3))
                elif i == 1:
                    nc.vector.tensor_mul(
                        osp[0:99, 0:qn], u_acc[0:99, qo : qo + qn], zrec[0:99, 0:qn]
                    )
                    for d in (0, 1):
                        otv = ot[0 : 2 * HLOC, qo : qo + qn].rearrange(
                            "(h g) f -> h g f", g=2
                        )[:, d : d + 1, :]
                        nc.sync.dma_start(out=otv, in_=hg(osp, 0, qn, d, d + 1))
                elif i == 2:
                    # transposed proj: one matmul, y^T layout [C, tokens]
                    nc.tensor.matmul(
                        PS[0:C, buf + 1024 : buf + 1024 + qn],
                        wp_sb[:],
                        ot[0 : 2 * HLOC + 1, qo : qo + qn],
                        start=True, stop=True,
                    )
                    nc.vector.tensor_copy(
                        ySB[:, qo : qo + qn], PS[0:C, buf + 1024 : buf + 1024 + qn]
                    )
                elif i == 3:
                    nc.sync.dma_start(
                        out=y[:, qo : qo + qn], in_=ySB[:, qo : qo + qn]
                    )

            def boundary(b, kc, buf):
                if b == 0:
                    if kc <= 9:  # k chunks 4..13 (last is 64 wide)
                        ko2, kw = KCS[kc + 4]
                        qk_piece(wk_sb, kT, ko2, kw, buf + 1024)
                    elif kc in (10, 11):  # q block1
                        qk_piece(wq_sb, qT, 512 + 256 * (kc - 10), 256, buf + 1024)
                else:
                    if 2 <= kc <= 5:
                        divide_piece(b - 1, kc - 2, buf)
                    if b == 1 and kc in (9, 10):  # q block2
                        qk_piece(wq_sb, qT, 1024 + 256 * (kc - 9), 256, buf + 1024)
                    if b == 2 and kc == 9:  # q block3 (192 wide)
                        qk_piece(wq_sb, qT, 1536, 192, buf + 1024)

            # ---- prologue PE burst (staged in bufB regions) ----
            qk_piece(wq_sb, qT, 0, 512, 2560)   # q block0 -> bufB bank1
            qk_piece(wk_sb, kT, 0, 128, 3584)   # k chunk0 -> bufB bank3
            emit_S(0, 0)

            # ---- main loop (S software-pipelined one chunk ahead) ----
            for b in range(4):
                for kc in range(NKC):
                    buf = 0 if kc % 2 == 0 else 2048
                    et = emit_exp(b, kc)
                    if (b, kc) == (0, 0):
                        # fill PE under the first exps: V' + k chunks 1-3
                        vprime(0, 7)
                        for i in range(1, 4):
                            qk_piece(wk_sb, kT, 128 * i, 128, 3584 + 128 * i)
                    if kc < 13:
                        emit_S(b, kc + 1)
                    elif b < 3:
                        emit_S(b + 1, 0)
                    emit_U_add(b, kc, et)
                    if (b, kc) == (0, 1):
                        vprime(7, 14)
                    boundary(b, kc, buf)

            # ---- tail: divide + proj + store for the final 192-block ----
            for i in range(4):
                divide_piece(3, i, 0)

    return nc


_NC = None


def _get_nc():
    global _NC
    if _NC is None:
        _NC = build_nc()
        _NC.finalize()
    return _NC


def make_in_maps(x, w_qkv, w_proj, b_proj):
    x2 = np.ascontiguousarray(x.reshape(C, N)).astype(ml_dtypes.bfloat16)
    in_maps = []
    for c in range(NCORES):
        sl = slice(8 * c, 8 * c + 8)
        wq = np.ascontiguousarray(w_qkv[sl, :].T).astype(ml_dtypes.bfloat16)
        wk = np.ascontiguousarray(w_qkv[64 + 8 * c : 64 + 8 * c + 8, :].T).astype(
            ml_dtypes.bfloat16
        )
        wv = np.ascontiguousarray(w_qkv[128 + 8 * c : 128 + 8 * c + 8, :].T).astype(
            ml_dtypes.bfloat16
        )
        wp = np.concatenate(
            [w_proj[:, sl].T, (b_proj / NCORES)[None, :]], axis=0
        ).astype(np.float32)
        in_maps.append(
            {"x2": x2, "wq": wq, "wk": wk, "wv": wv, "wp": np.ascontiguousarray(wp)}
        )
    return in_maps


def run(x, w_qkv, w_proj, b_proj, trace=False, **kw):
    nc = _get_nc()
    in_maps = make_in_maps(x, w_qkv, w_proj, b_proj)
    res = run_bass_kernel_spmd(
        nc, in_maps, core_ids=list(range(NCORES)), trace=trace, **kw
    )
    y = np.zeros((C, N), np.float32)
    for r in res.results:
        y += r["y"]
    return np.ascontiguousarray(y.T).reshape(1, 12, 12, 12, C), res


def kernel(x, w_qkv, w_proj, b_proj):
    out, _ = run(
        np.asarray(x), np.asarray(w_qkv), np.asarray(w_proj), np.asarray(b_proj)
    )
    return out


# revision 31
# speedup vs baseline: 1.3870x; 1.0141x over previous
"""Trainium2 Bass kernel for nn_Attention (B=1, C=64, 12x12x12 spatial, 32 heads, head_dim=2).

Sharding: 32 heads split across 8 cores (4 heads/core), host sums the
8 partial w_proj outputs (tensor-parallel unshard, bias/8 per core).

Core design (v2, ACT-bound at ~91us of exp):
- Query blocks of (512,512,512,192), key chunks 13x128 + 64.
- One exp ACTIVATE per (block, chunk) covering all 4 heads (F=2048 from
  4 PSUM banks) -> amortizes the ~290-cycle per-instruction ACT overhead.
- PSUM managed manually as one [128,4096] tile: two 4-bank S buffers
  ping-pong (even/odd chunk). U_chunk matmuls are carved into bank 3 of
  the buffer ACT just finished (h3's region -- the LAST S matmul of the
  next same-parity chunk to touch it, so the DVE drain hides), proj and
  qkv staging into bank 2. DVE accumulates U into SBUF (u_acc) so no
  PSUM bank persists across the chunk loop.
- Software pipelining: S(kc+1) is emitted BEFORE U(kc) -- PE's queue is
  strict in-order, so the baseline's order (U before next S) serialized
  exp(kc) -> U(kc) -> S(kc+1) -> exp(kc+1) and starved ACT.
- Tail key chunk (64 keys) packs head pairs on partitions (rows 0:64 /
  64:128) halving its exp free-size; U uses block-diagonal V' weights.
- Per-head qkv matmuls run as a dense PE burst at t0 (warms the PE HAM
  clock gate; cold 1.2GHz PE was half the baseline's loss) and continue
  as carved pieces at chunk boundaries.
- Dummy 8-elem exp at t0 pulls the ~2.7us ACT table load under the
  input DMA. Divide uses reciprocal_approx_fast; divide+proj of block b
  run under block b+1's chunk loop so only the 192-block drains at the
  end.
"""

import numpy as np
import ml_dtypes

import concourse.bass as bass
import concourse.bacc as bacc
import concourse.mybir as mybir
from concourse import tile
from concourse.bass_utils import run_bass_kernel_spmd

C = 64
N = 1728
NCORES = 8
HLOC = 4
SCALE = float(2.0 ** -0.5)

KCS = [(i * 128, 128) for i in range(13)] + [(1664, 64)]
NKC = len(KCS)
QB = [(0, 512), (512, 512), (1024, 512), (1536, 192)]

F32 = mybir.dt.float32
BF16 = mybir.dt.bfloat16
EXPF = mybir.ActivationFunctionType.Exp


def build_nc():
    nc = bacc.Bacc(None)

    x2 = nc.declare_dram_parameter("x2", [C, N], BF16, isOutput=False)
    wq = nc.declare_dram_parameter("wq", [C, 2 * HLOC], BF16, isOutput=False)
    wk = nc.declare_dram_parameter("wk", [C, 2 * HLOC], BF16, isOutput=False)
    wv = nc.declare_dram_parameter("wv", [C, 2 * HLOC], BF16, isOutput=False)
    wp = nc.declare_dram_parameter("wp", [2 * HLOC + 1, C], F32, isOutput=False)
    y = nc.declare_dram_parameter("y", [C, N], F32, isOutput=True)

    with tile.TileContext(nc) as tc:
        with (
            tc.tile_pool(name="const", bufs=1) as cpool,
            tc.tile_pool(name="epool", bufs=3) as epool,
            tc.tile_pool(name="ps", bufs=1, space=bass.MemorySpace.PSUM) as pspool,
        ):
            x_sb = cpool.tile([C, N], BF16, name="x_sb")
            wq_sb = cpool.tile([C, 2 * HLOC], BF16, name="wq_sb")
            wk_sb = cpool.tile([C, 2 * HLOC], BF16, name="wk_sb")
            wv_sb = cpool.tile([C, 2 * HLOC], BF16, name="wv_sb")
            wp_sb = cpool.tile([2 * HLOC + 1, C], F32, name="wp_sb")
            qT = cpool.tile([128, N], BF16, name="qT")
            kT = cpool.tile([128, N], BF16, name="kT")
            vp = cpool.tile([128, NKC * HLOC * 3], BF16, name="vp")
            u_acc = cpool.tile([128, N], F32, name="u_acc")
            zot = cpool.tile([16, 512], F32, name="zot")
            zotr = cpool.tile([16, 512], F32, name="zotr")
            ot = cpool.tile([16, N], F32, name="ot")
            junk = cpool.tile([C, 1024], BF16, name="junk")
            ySB = cpool.tile([C, N], F32, name="ySB")
            dum = cpool.tile([1, 16], F32, name="dum")
            PS = pspool.tile([128, 4096], F32, name="PS")

            vp_v = vp[:].rearrange("p (kc h d) -> p kc h d", h=HLOC, d=3)

            def hg(t, c0, c1, r0, r1, g=32):
                """Partitions {g*h + r0..r1}, cols c0..c1 -> [4, r, c] view."""
                return t[:, c0:c1].rearrange("(h g) f -> h g f", g=g)[:, r0:r1, :]

            # ---- t0: ACT table prefetch first (nothing on the ACT queue
            # before the dummy exp), input DMAs on sync (spread over HW
            # queues), weights on gpsimd SWDGE ----
            nc.vector.memset(dum[:], 1.0)
            nc.scalar.activation(dum[0:1, 8:16], dum[0:1, 0:8], EXPF)
            nc.sync.dma_start(out=x_sb[:, 0:576], in_=x2[:, 0:576])
            nc.sync.dma_start(out=x_sb[:, 576:1152], in_=x2[:, 576:1152])
            nc.sync.dma_start(out=x_sb[:, 1152:N], in_=x2[:, 1152:N])
            nc.gpsimd.dma_start(out=wq_sb[:], in_=wq[:])
            nc.gpsimd.dma_start(out=wk_sb[:], in_=wk[:])
            nc.gpsimd.dma_start(out=wv_sb[:], in_=wv[:])
            nc.gpsimd.dma_start(out=wp_sb[:], in_=wp[:])
            nc.gpsimd.memset(ot[:, :], 1.0)
            nc.gpsimd.memset(vp_v[:, :, :, 2:3], 1.0)
            # warm the PE pipeline + HAM clock gate while x is in flight
            nc.vector.memset(junk[:, :], 0.5)
            for i in range(3):
                nc.tensor.matmul(
                    PS[0:2, 2048 + 512 * (i % 2) : 2560 + 512 * (i % 2)],
                    junk[:, 0:2], junk[:, 2:514],
                    start=True, stop=True,
                )

            # ---- helpers ----
            def qk_piece(w_sb, dst, o, w, creg):
                """q or k for all 4 heads over x cols [o, o+w) via psum carve."""
                for h in range(HLOC):
                    nc.tensor.matmul(
                        PS[32 * h : 32 * h + 2, creg : creg + w],
                        w_sb[:, 2 * h : 2 * h + 2],
                        x_sb[:, o : o + w],
                        start=True, stop=True,
                        tile_position=(0, 32 * h),
                    )
                # contiguous partitions (DVE can't stride the partition dim);
                # junk rows between head groups land in unused qT/kT rows
                nc.vector.tensor_copy(
                    dst[0:98, o : o + w], PS[0:98, creg : creg + w]
                )

            def vprime(k0, k1, base):
                for kc in range(k0, k1):
                    ko, kn = KCS[kc]
                    o = base + 8 * (kc - k0)
                    nc.tensor.matmul(
                        PS[0:kn, o : o + 8],
                        x_sb[:, ko : ko + kn],
                        wv_sb[:, 0 : 2 * HLOC],
                        start=True, stop=True,
                    )
                vsrc = PS[:, base : base + 8 * (k1 - k0)].rearrange(
                    "p (kc h d) -> p kc h d", h=HLOC, d=2
                )
                nc.vector.tensor_copy(vp_v[:, k0:k1, :, 0:2], vsrc)

            def emit_S(b, kc):
                qo, qn = QB[b]
                ko, kn = KCS[kc]
                buf = 0 if kc % 2 == 0 else 2048
                for h in range(HLOC):
                    nc.tensor.matmul(
                        PS[0:kn, buf + 512 * h : buf + 512 * h + qn],
                        kT[32 * h : 32 * h + 2, ko : ko + kn],
                        qT[32 * h : 32 * h + 2, qo : qo + qn],
                        start=True, stop=True,
                        tile_position=(32 * h, 0),
                    )

            def emit_exp(b, kc):
                qo, qn = QB[b]
                ko, kn = KCS[kc]
                buf = 0 if kc % 2 == 0 else 2048
                et = epool.tile([128, 2048], BF16, tag="e", name="et")
                if qn == 512:
                    nc.scalar.activation(
                        et[0:kn, 0:2048], PS[0:kn, buf : buf + 2048],
                        EXPF, scale=SCALE,
                    )
                else:
                    src = PS[0:kn, buf : buf + 2048].rearrange(
                        "p (h q) -> p h q", h=4
                    )[:, :, 0:qn]
                    dst = et[0:kn, 0 : 4 * qn].rearrange("p (h q) -> p h q", h=4)
                    nc.scalar.activation(dst, src, EXPF, scale=SCALE)
                return et

            def emit_U_add(b, kc, et):
                qo, qn = QB[b]
                ko, kn = KCS[kc]
                buf = 0 if kc % 2 == 0 else 2048
                cv = buf + 1536  # carve: bank 3 of the freed buffer
                for h in range(HLOC):
                    nc.tensor.matmul(
                        PS[32 * h : 32 * h + 3, cv : cv + qn],
                        vp_v[0:kn, kc, h, :],
                        et[0:kn, qn * h : qn * h + qn],
                        start=True, stop=True,
                        tile_position=(0, 32 * h),
                    )
                uc = PS[0:99, cv : cv + qn]
                ua = u_acc[0:99, qo : qo + qn]
                if kc == 0:
                    nc.vector.tensor_copy(ua, uc)
                else:
                    nc.vector.tensor_add(ua, ua, uc)

            def divide_piece(bprev, i, buf):
                qo, qn = QB[bprev]

                if i == 0:
                    # one parallel DMA wave: scatter U rows {32h+d} -> ot rows
                    # {2h+d}, and Z rows {32h+2} -> zot rows {2h+d}
                    for d, eng in ((0, nc.sync), (1, nc.gpsimd)):
                        otv = ot[0 : 2 * HLOC, qo : qo + qn].rearrange(
                            "(h g) f -> h g f", g=2
                        )[:, d : d + 1, :]
                        eng.dma_start(out=otv, in_=hg(u_acc, qo, qo + qn, d, d + 1))
                        ztv = zot[0 : 2 * HLOC, 0:qn].rearrange(
                            "(h g) f -> h g f", g=2
                        )[:, d : d + 1, :]
                        eng.dma_start(out=ztv, in_=hg(u_acc, qo, qo + qn, 2, 3))
                elif i == 1:
                    nc.vector.reciprocal_approx_fast(
                        zotr[0 : 2 * HLOC, 0:qn], zot[0 : 2 * HLOC, 0:qn]
                    )
                elif i == 2:
                    nc.vector.tensor_mul(
                        ot[0 : 2 * HLOC, qo : qo + qn],
                        ot[0 : 2 * HLOC, qo : qo + qn],
                        zotr[0 : 2 * HLOC, 0:qn],
                    )
                elif i == 3:
                    # transposed proj: one matmul, y^T layout [C, tokens]
                    nc.tensor.matmul(
                        PS[0:C, buf + 1024 : buf + 1024 + qn],
                        wp_sb[:],
                        ot[0 : 2 * HLOC + 1, qo : qo + qn],
                        start=True, stop=True,
                    )
                    nc.vector.tensor_copy(
                        ySB[:, qo : qo + qn], PS[0:C, buf + 1024 : buf + 1024 + qn]
                    )
                elif i == 4:
                    nc.sync.dma_start(
                        out=y[:, qo : qo + qn], in_=ySB[:, qo : qo + qn]
                    )

            def boundary(b, kc, buf):
                if b == 0:
                    if kc <= 9:  # k chunks 4..13 (last is 64 wide)
                        ko2, kw = KCS[kc + 4]
                        qk_piece(wk_sb, kT, ko2, kw, buf + 1024)
                    elif kc in (10, 11):  # q block1
                        qk_piece(wq_sb, qT, 512 + 256 * (kc - 10), 256, buf + 1024)
                else:
                    if 2 <= kc <= 6:
                        divide_piece(b - 1, kc - 2, buf)
                    if b == 1 and kc in (9, 10):  # q block2
                        qk_piece(wq_sb, qT, 1024 + 256 * (kc - 9), 256, buf + 1024)
                    if b == 2 and kc == 9:  # q block3 (192 wide)
                        qk_piece(wq_sb, qT, 1536, 192, buf + 1024)

            # ---- prologue PE burst (staged in bufB regions) ----
            qk_piece(wq_sb, qT, 0, 512, 2560)   # q block0 -> bufB bank1
            qk_piece(wk_sb, kT, 0, 128, 3584)   # k chunk0 -> bufB bank3
            emit_S(0, 0)

            # ---- main loop (S software-pipelined one chunk ahead) ----
            for b in range(4):
                for kc in range(NKC):
                    buf = 0 if kc % 2 == 0 else 2048
                    et = emit_exp(b, kc)
                    if (b, kc) == (0, 0):
                        # fill PE under the first exps: V' piece + k chunks 1-2
                        vprime(0, 4, 2048)
                        qk_piece(wk_sb, kT, 128, 128, 3712)
                        qk_piece(wk_sb, kT, 256, 128, 3840)
                    if kc < 13:
                        emit_S(b, kc + 1)
                    elif b < 3:
                        emit_S(b + 1, 0)
                    emit_U_add(b, kc, et)
                    if (b, kc) == (0, 1):
                        vprime(4, 9, 2080)
                        qk_piece(wk_sb, kT, 384, 128, 3968)
                    elif (b, kc) == (0, 2):
                        vprime(9, 14, 1200)
                    boundary(b, kc, buf)

            # ---- tail: divide + proj + store for the final 192-block ----
            for i in range(5):
                divide_piece(3, i, 0)

    return nc


_NC = None


def _get_nc():
    global _NC
    if _NC is None:
        _NC = build_nc()
        _NC.finalize()
    return _NC


def make_in_maps(x, w_qkv, w_proj, b_proj):
    x2 = np.ascontiguousarray(x.reshape(C, N)).astype(ml_dtypes.bfloat16)
    in_maps = []
    for c in range(NCORES):
        sl = slice(8 * c, 8 * c + 8)
        wq = np.ascontiguousarray(w_qkv[sl, :].T).astype(ml_dtypes.bfloat16)
        wk = np.ascontiguousarray(w_qkv[64 + 8 * c : 64 + 8 * c + 8, :].T).astype(
            ml_dtypes.bfloat16
        )
        wv = np.ascontiguousarray(w_qkv[128 + 8 * c : 128 + 8 * c + 8, :].T).astype(
            ml_dtypes.bfloat16
        )
        wp = np.concatenate(
            [w_proj[:, sl].T, (b_proj / NCORES)[None, :]], axis=0
        ).astype(np.float32)
        in_maps.append(
            {"x2": x2, "wq": wq, "wk": wk, "wv": wv, "wp": np.ascontiguousarray(wp)}
        )
    return in_maps


def run(x, w_qkv, w_proj, b_proj, trace=False, **kw):
    nc = _get_nc()
    in_maps = make_in_maps(x, w_qkv, w_proj, b_proj)
    res = run_bass_kernel_spmd(
        nc, in_maps, core_ids=list(range(NCORES)), trace=trace, **kw
    )
    y = np.zeros((C, N), np.float32)
    for r in res.results:
        y += r["y"]
    return np.ascontiguousarray(y.T).reshape(1, 12, 12, 12, C), res


def kernel(x, w_qkv, w_proj, b_proj):
    out, _ = run(
        np.asarray(x), np.asarray(w_qkv), np.asarray(w_proj), np.asarray(b_proj)
    )
    return out


# revision 38
# speedup vs baseline: 1.4193x; 1.0233x over previous
"""Trainium2 Bass kernel for nn_Attention (B=1, C=64, 12x12x12 spatial, 32 heads, head_dim=2).

Sharding: 32 heads split across 8 cores (4 heads/core), host sums the
8 partial w_proj outputs (tensor-parallel unshard, bias/8 per core).

Core design (v2, ACT-bound at ~91us of exp):
- Query blocks of (512,512,512,192), key chunks 13x128 + 64.
- One exp ACTIVATE per (block, chunk) covering all 4 heads (F=2048 from
  4 PSUM banks) -> amortizes the ~290-cycle per-instruction ACT overhead.
- PSUM managed manually as one [128,4096] tile: two 4-bank S buffers
  ping-pong (even/odd chunk). U_chunk matmuls are carved into bank 3 of
  the buffer ACT just finished (h3's region -- the LAST S matmul of the
  next same-parity chunk to touch it, so the DVE drain hides), proj and
  qkv staging into bank 2. DVE accumulates U into SBUF (u_acc) so no
  PSUM bank persists across the chunk loop.
- Software pipelining: S(kc+1) is emitted BEFORE U(kc) -- PE's queue is
  strict in-order, so the baseline's order (U before next S) serialized
  exp(kc) -> U(kc) -> S(kc+1) -> exp(kc+1) and starved ACT.
- Tail key chunk (64 keys) packs head pairs on partitions (rows 0:64 /
  64:128) halving its exp free-size; U uses block-diagonal V' weights.
- Per-head qkv matmuls run as a dense PE burst at t0 (warms the PE HAM
  clock gate; cold 1.2GHz PE was half the baseline's loss) and continue
  as carved pieces at chunk boundaries.
- Dummy 8-elem exp at t0 pulls the ~2.7us ACT table load under the
  input DMA. Divide uses reciprocal_approx_fast; divide+proj of block b
  run under block b+1's chunk loop so only the 192-block drains at the
  end.
"""

import numpy as np
import ml_dtypes

import concourse.bass as bass
import concourse.bacc as bacc
import concourse.mybir as mybir
from concourse import tile
from concourse.bass_utils import run_bass_kernel_spmd

C = 64
N = 1728
NCORES = 8
HLOC = 4
SCALE = float(2.0 ** -0.5)

KCS = [(i * 128, 128) for i in range(13)] + [(1664, 64)]
NKC = len(KCS)
QB = [(0, 512), (512, 512), (1024, 512), (1536, 192)]

F32 = mybir.dt.float32
BF16 = mybir.dt.bfloat16
EXPF = mybir.ActivationFunctionType.Exp


def build_nc():
    nc = bacc.Bacc(None)

    x2 = nc.declare_dram_parameter("x2", [C, N], BF16, isOutput=False)
    wq = nc.declare_dram_parameter("wq", [C, 2 * HLOC], BF16, isOutput=False)
    wk = nc.declare_dram_parameter("wk", [C, 2 * HLOC], BF16, isOutput=False)
    wv = nc.declare_dram_parameter("wv", [C, 2 * HLOC], BF16, isOutput=False)
    wp = nc.declare_dram_parameter("wp", [2 * HLOC + 1, C], F32, isOutput=False)
    y = nc.declare_dram_parameter("y", [C, N], F32, isOutput=True)

    with tile.TileContext(nc) as tc:
        with (
            tc.tile_pool(name="const", bufs=1) as cpool,
            tc.tile_pool(name="epool", bufs=3) as epool,
            tc.tile_pool(name="ps", bufs=1, space=bass.MemorySpace.PSUM) as pspool,
        ):
            x_sb = cpool.tile([C, N], BF16, name="x_sb")
            wq_sb = cpool.tile([C, 2 * HLOC], BF16, name="wq_sb")
            wk_sb = cpool.tile([C, 2 * HLOC], BF16, name="wk_sb")
            wv_sb = cpool.tile([C, 2 * HLOC], BF16, name="wv_sb")
            wp_sb = cpool.tile([2 * HLOC + 1, C], F32, name="wp_sb")
            qT = cpool.tile([128, N], BF16, name="qT")
            kT = cpool.tile([128, N], BF16, name="kT")
            qst = cpool.tile([2 * HLOC, N], BF16, name="qst")
            kst = cpool.tile([2 * HLOC, N], BF16, name="kst")
            vp = cpool.tile([128, NKC * HLOC * 3], BF16, name="vp")
            u_acc = cpool.tile([128, N], F32, name="u_acc")
            zot = cpool.tile([16, 512], F32, name="zot")
            zotr = cpool.tile([16, 512], F32, name="zotr")
            ot = cpool.tile([16, N], F32, name="ot")
            junk = cpool.tile([C, 1024], BF16, name="junk")
            ySB = cpool.tile([C, N], F32, name="ySB")
            dum = cpool.tile([1, 16], F32, name="dum")
            PS = pspool.tile([128, 4096], F32, name="PS")

            vp_v = vp[:].rearrange("p (kc h d) -> p kc h d", h=HLOC, d=3)

            def hg(t, c0, c1, r0, r1, g=32):
                """Partitions {g*h + r0..r1}, cols c0..c1 -> [4, r, c] view."""
                return t[:, c0:c1].rearrange("(h g) f -> h g f", g=g)[:, r0:r1, :]

            # ---- t0: ACT table prefetch first (nothing on the ACT queue
            # before the dummy exp), input DMAs on sync (spread over HW
            # queues), weights on gpsimd SWDGE ----
            nc.vector.memset(dum[:], 1.0)
            nc.scalar.activation(dum[0:1, 8:16], dum[0:1, 0:8], EXPF)
            nc.sync.dma_start(out=x_sb[:, 0:576], in_=x2[:, 0:576])
            nc.sync.dma_start(out=x_sb[:, 576:1152], in_=x2[:, 576:1152])
            nc.sync.dma_start(out=x_sb[:, 1152:N], in_=x2[:, 1152:N])
            nc.gpsimd.dma_start(out=wq_sb[:], in_=wq[:])
            nc.gpsimd.dma_start(out=wk_sb[:], in_=wk[:])
            nc.gpsimd.dma_start(out=wv_sb[:], in_=wv[:])
            nc.gpsimd.dma_start(out=wp_sb[:], in_=wp[:])
            nc.gpsimd.memset(ot[:, :], 1.0)
            nc.gpsimd.memset(vp_v[:, :, :, 2:3], 1.0)
            # warm the PE pipeline + HAM clock gate while x is in flight
            nc.vector.memset(junk[:, :], 0.5)
            for i in range(3):
                nc.tensor.matmul(
                    PS[0:2, 2048 + 512 * (i % 2) : 2560 + 512 * (i % 2)],
                    junk[:, 0:2], junk[:, 2:514],
                    start=True, stop=True,
                )

            # ---- helpers ----
            def qk_piece(w_sb, dst, o, w, creg):
                """q or k for all 4 heads over x cols [o, o+w) via psum carve."""
                for h in range(HLOC):
                    nc.tensor.matmul(
                        PS[32 * h : 32 * h + 2, creg : creg + w],
                        w_sb[:, 2 * h : 2 * h + 2],
                        x_sb[:, o : o + w],
                        start=True, stop=True,
                        tile_position=(0, 32 * h),
                    )
                # contiguous partitions (DVE can't stride the partition dim);
                # junk rows between head groups land in unused qT/kT rows
                nc.vector.tensor_copy(
                    dst[0:98, o : o + w], PS[0:98, creg : creg + w]
                )

            def packed_piece(w_sb, stage, dstT, o, w, creg):
                """q or k for all heads in ONE matmul (out partitions 0:8 =
                (h,d)), then DVE->SBUF stage and DMA scatter into the
                32h-strided layout. 1 PE instruction instead of 4."""
                nc.tensor.matmul(
                    PS[0 : 2 * HLOC, creg : creg + w],
                    w_sb[:, 0 : 2 * HLOC],
                    x_sb[:, o : o + w],
                    start=True, stop=True,
                )
                nc.vector.tensor_copy(
                    stage[:, o : o + w], PS[0 : 2 * HLOC, creg : creg + w]
                )
                for d, eng in ((0, nc.sync), (1, nc.gpsimd)):
                    eng.dma_start(
                        out=dstT[:, o : o + w].rearrange("(h g) t -> h g t", g=32)[
                            :, d : d + 1, :
                        ],
                        in_=stage[:, o : o + w].rearrange("(h d) t -> h d t", d=2)[
                            :, d : d + 1, :
                        ],
                    )

            def vprime(k0, k1, base):
                for kc in range(k0, k1):
                    ko, kn = KCS[kc]
                    o = base + 8 * (kc - k0)
                    nc.tensor.matmul(
                        PS[0:kn, o : o + 8],
                        x_sb[:, ko : ko + kn],
                        wv_sb[:, 0 : 2 * HLOC],
                        start=True, stop=True,
                    )
                vsrc = PS[:, base : base + 8 * (k1 - k0)].rearrange(
                    "p (kc h d) -> p kc h d", h=HLOC, d=2
                )
                nc.vector.tensor_copy(vp_v[:, k0:k1, :, 0:2], vsrc)

            def emit_S(b, kc):
                qo, qn = QB[b]
                ko, kn = KCS[kc]
                buf = 0 if kc % 2 == 0 else 2048
                for h in range(HLOC):
                    nc.tensor.matmul(
                        PS[0:kn, buf + 512 * h : buf + 512 * h + qn],
                        kT[32 * h : 32 * h + 2, ko : ko + kn],
                        qT[32 * h : 32 * h + 2, qo : qo + qn],
                        start=True, stop=True,
                        tile_position=(32 * h, 0),
                    )

            def emit_exp(b, kc):
                qo, qn = QB[b]
                ko, kn = KCS[kc]
                buf = 0 if kc % 2 == 0 else 2048
                et = epool.tile([128, 2048], BF16, tag="e", name="et")
                if qn == 512:
                    nc.scalar.activation(
                        et[0:kn, 0:2048], PS[0:kn, buf : buf + 2048],
                        EXPF, scale=SCALE,
                    )
                else:
                    src = PS[0:kn, buf : buf + 2048].rearrange(
                        "p (h q) -> p h q", h=4
                    )[:, :, 0:qn]
                    dst = et[0:kn, 0 : 4 * qn].rearrange("p (h q) -> p h q", h=4)
                    nc.scalar.activation(dst, src, EXPF, scale=SCALE)
                return et

            def emit_U_add(b, kc, et):
                qo, qn = QB[b]
                ko, kn = KCS[kc]
                buf = 0 if kc % 2 == 0 else 2048
                cv = buf + 1536  # carve: bank 3 of the freed buffer
                for h in range(HLOC):
                    nc.tensor.matmul(
                        PS[32 * h : 32 * h + 3, cv : cv + qn],
                        vp_v[0:kn, kc, h, :],
                        et[0:kn, qn * h : qn * h + qn],
                        start=True, stop=True,
                        tile_position=(0, 32 * h),
                    )
                uc = PS[0:99, cv : cv + qn]
                ua = u_acc[0:99, qo : qo + qn]
                if kc == 0:
                    nc.vector.tensor_copy(ua, uc)
                else:
                    nc.vector.tensor_add(ua, ua, uc)

            def divide_piece(bprev, i, buf):
                qo, qn = QB[bprev]

                if i == 0:
                    # one parallel DMA wave: scatter U rows {32h+d} -> ot rows
                    # {2h+d}, and Z rows {32h+2} -> zot rows {2h+d}
                    for d, eng in ((0, nc.sync), (1, nc.gpsimd)):
                        otv = ot[0 : 2 * HLOC, qo : qo + qn].rearrange(
                            "(h g) f -> h g f", g=2
                        )[:, d : d + 1, :]
                        eng.dma_start(out=otv, in_=hg(u_acc, qo, qo + qn, d, d + 1))
                        ztv = zot[0 : 2 * HLOC, 0:qn].rearrange(
                            "(h g) f -> h g f", g=2
                        )[:, d : d + 1, :]
                        eng.dma_start(out=ztv, in_=hg(u_acc, qo, qo + qn, 2, 3))
                elif i == 1:
                    nc.vector.reciprocal_approx_fast(
                        zotr[0 : 2 * HLOC, 0:qn], zot[0 : 2 * HLOC, 0:qn]
                    )
                elif i == 2:
                    nc.vector.tensor_mul(
                        ot[0 : 2 * HLOC, qo : qo + qn],
                        ot[0 : 2 * HLOC, qo : qo + qn],
                        zotr[0 : 2 * HLOC, 0:qn],
                    )
                elif i == 3:
                    # transposed proj: one matmul, y^T layout [C, tokens]
                    nc.tensor.matmul(
                        PS[0:C, buf + 1024 : buf + 1024 + qn],
                        wp_sb[:],
                        ot[0 : 2 * HLOC + 1, qo : qo + qn],
                        start=True, stop=True,
                    )
                    nc.vector.tensor_copy(
                        ySB[:, qo : qo + qn], PS[0:C, buf + 1024 : buf + 1024 + qn]
                    )
                elif i == 4:
                    nc.sync.dma_start(
                        out=y[:, qo : qo + qn], in_=ySB[:, qo : qo + qn]
                    )

            def boundary(b, kc, buf):
                if b > 0 and 2 <= kc <= 6:
                    divide_piece(b - 1, kc - 2, buf)

            # ---- prologue PE burst (staged in bufB regions) ----
            qk_piece(wq_sb, qT, 0, 512, 2560)       # q block0 -> bufB bank1
            qk_piece(wk_sb, kT, 0, 128, 3584)       # k chunk0 -> bufB bank3
            packed_piece(wk_sb, kst, kT, 128, 512, 3072)  # k chunks 1-4
            emit_S(0, 0)

            # ---- main loop (S software-pipelined one chunk ahead) ----
            for b in range(4):
                for kc in range(NKC):
                    buf = 0 if kc % 2 == 0 else 2048
                    et = emit_exp(b, kc)
                    if (b, kc) == (0, 0):
                        # fill PE under the first exps: V' piece + k chunks 5-8
                        vprime(0, 4, 2048)
                        packed_piece(wk_sb, kst, kT, 640, 512, 3072)
                    if kc < 13:
                        emit_S(b, kc + 1)
                    elif b < 3:
                        emit_S(b + 1, 0)
                    emit_U_add(b, kc, et)
                    if b == 0:
                        if kc == 1:
                            vprime(4, 8, 2048)
                            packed_piece(wk_sb, kst, kT, 1152, 512, 3072)
                        elif kc == 2:
                            vprime(8, 11, 0)
                            packed_piece(wk_sb, kst, kT, 1664, 64, 1024)
                        elif kc == 3:
                            vprime(11, 14, 2048)
                            packed_piece(wq_sb, qst, qT, 512, 512, 3072)
                        elif kc == 5:
                            packed_piece(wq_sb, qst, qT, 1024, 512, 3072)
                        elif kc == 7:
                            packed_piece(wq_sb, qst, qT, 1536, 192, 3072)
                    boundary(b, kc, buf)

            # ---- tail: divide + proj + store for the final 192-block ----
            for i in range(5):
                divide_piece(3, i, 0)

    return nc


_NC = None


def _get_nc():
    global _NC
    if _NC is None:
        _NC = build_nc()
        _NC.finalize()
    return _NC


def make_in_maps(x, w_qkv, w_proj, b_proj):
    x2 = np.ascontiguousarray(x.reshape(C, N)).astype(ml_dtypes.bfloat16)
    in_maps = []
    for c in range(NCORES):
        sl = slice(8 * c, 8 * c + 8)
        wq = np.ascontiguousarray(w_qkv[sl, :].T).astype(ml_dtypes.bfloat16)
        wk = np.ascontiguousarray(w_qkv[64 + 8 * c : 64 + 8 * c + 8, :].T).astype(
            ml_dtypes.bfloat16
        )
        wv = np.ascontiguousarray(w_qkv[128 + 8 * c : 128 + 8 * c + 8, :].T).astype(
            ml_dtypes.bfloat16
        )
        wp = np.concatenate(
            [w_proj[:, sl].T, (b_proj / NCORES)[None, :]], axis=0
        ).astype(np.float32)
        in_maps.append(
            {"x2": x2, "wq": wq, "wk": wk, "wv": wv, "wp": np.ascontiguousarray(wp)}
        )
    return in_maps


def run(x, w_qkv, w_proj, b_proj, trace=False, **kw):
    nc = _get_nc()
    in_maps = make_in_maps(x, w_qkv, w_proj, b_proj)
    res = run_bass_kernel_spmd(
        nc, in_maps, core_ids=list(range(NCORES)), trace=trace, **kw
    )
    y = np.zeros((C, N), np.float32)
    for r in res.results:
        y += r["y"]
    return np.ascontiguousarray(y.T).reshape(1, 12, 12, 12, C), res


def kernel(x, w_qkv, w_proj, b_proj):
    out, _ = run(
        np.asarray(x), np.asarray(w_qkv), np.asarray(w_proj), np.asarray(b_proj)
    )
    return out
